# revision 1
# baseline (speedup 1.0000x reference)
"""DiffusionTransformerLayer on 8 Trainium2 NeuronCores.

Sharding: rows (B*N = 2048 tokens) split 256/core; attention K/V all-gathered
within each batch's 4-core group (one 4-rank AllGather of bf16 k^T/v).

Layout strategy:
  - "natural" activations: [rows(part), feat(free)]; LN / gates / residuals here.
  - matmul inputs are transposed on the PE (128x128 transposes) since the
    tensor engine contracts over the partition axis.
  - attention runs feature-on-partition: qT/kT produced directly by projecting
    with host-padded weights (heads padded 48->64 so each head sits at
    partition base 0/64 of an SBUF tile). Scores are computed transposed
    (ST[k,q]), softmax over k becomes: exp on ACT, pair-bias via elementwise
    multiply with host-precomputed exp(z^T), and the row-sum arrives free as a
    ones-column in the padded V weights through the P@V matmul.
  - all matmul operands bf16 (fp32 PSUM accumulation); residual stream fp32.
"""
import os

import numpy as np
import ml_dtypes

import concourse.bacc as bacc
import concourse.bass as bass
import concourse.tile as tile
from concourse import mybir
from concourse.bass_utils import run_bass_kernel_spmd

F32 = mybir.dt.float32
BF16 = mybir.dt.bfloat16
AF = mybir.ActivationFunctionType
OP = mybir.AluOpType
NPBF16 = ml_dtypes.bfloat16

B, N, D, H = 2, 1024, 768, 16
HD, HDP = 48, 64
HP = H * HDP          # 1024 padded head dims
HID = 1536
R = 256               # rows per core
FC = D // 128         # 6 feature chunks
EPS = 1e-5
SCALE = HD ** -0.5
KT_ELEMS = HP * R     # kT shard elems
V_ELEMS = R * HP      # v shard elems
SHARD = KT_ELEMS + V_ELEMS

_NC_CACHE = {}


def _build_nc():
    stage = int(os.environ.get("KSTAGE", "4"))
    nc = bacc.Bacc("TRN2", target_bir_lowering=False, debug=False, num_devices=8)

    din = {}

    def inp(name, shape, dt):
        din[name] = nc.dram_tensor(name, list(shape), dt, kind="ExternalInput")
        return din[name]

    a_in = inp("a_in", (R, D), F32)
    s_in = inp("s_in", (R, D), BF16)
    ez_in = inp("ez_in", (H, 128, 8, R), BF16)
    ident_in = inp("ident_in", (128, 128), BF16)
    sel_in = inp("sel_in", (4, 4, 256), BF16)
    vones_in = inp("vones_in", (1, HP), BF16)
    qb_in = inp("qb_in", (8, 128), F32)
    w_ss1 = inp("w_ss1", (D + 1, D), BF16)
    w_sb1 = inp("w_sb1", (D, D), BF16)
    w_ss2 = inp("w_ss2", (D + 1, D), BF16)
    w_sb2 = inp("w_sb2", (D, D), BF16)
    w_q = inp("w_q", (D, HP), BF16)
    w_k = inp("w_k", (D, HP), BF16)
    w_g = inp("w_g", (D, HP), BF16)
    w_v = inp("w_v", (D, HP), BF16)
    w_ow = inp("w_ow", (HP, D), BF16)
    w_op1 = inp("w_op1", (D + 1, D), BF16)
    w_op2 = inp("w_op2", (D + 1, D), BF16)
    w_swu = inp("w_swu", (D, HID), BF16)
    w_swg = inp("w_swg", (D, HID), BF16)
    w_ab = inp("w_ab", (D, HID), BF16)
    w_ba = inp("w_ba", (HID, D), BF16)

    y_out = nc.dram_tensor("y", [R, D], F32, kind="ExternalOutput")

    with tile.TileContext(nc) as tc:
        from contextlib import ExitStack
        with ExitStack() as ctx:
            cst = ctx.enter_context(tc.tile_pool(name="cst", bufs=1))
            acts = ctx.enter_context(tc.tile_pool(name="acts", bufs=1))
            lnp = ctx.enter_context(tc.tile_pool(name="lnp", bufs=3))
            wp = ctx.enter_context(tc.tile_pool(name="wp", bufs=4))
            wcb = ctx.enter_context(tc.tile_pool(name="wcb", bufs=1))
            ezp = ctx.enter_context(tc.tile_pool(name="ezp", bufs=4))
            pp = ctx.enter_context(tc.tile_pool(name="pp", bufs=5))
            tmp = ctx.enter_context(tc.tile_pool(name="tmp", bufs=2))
            dram = ctx.enter_context(tc.tile_pool(name="dram", bufs=1, space="DRAM"))
            ps2 = ctx.enter_context(tc.tile_pool(name="ps2", bufs=2, space="PSUM"))
            ps1 = ctx.enter_context(tc.tile_pool(name="ps1", bufs=4, space="PSUM"))

            # ---------------- load s (+ident) first: feeds the first PE work --
            a_sb, s_sb = [], []
            for rt in range(2):
                st = acts.tile([128, D], BF16, tag=f"s{rt}")
                nc.sync.dma_start(out=st, in_=s_in.ap()[128 * rt:128 * (rt + 1), :])
                s_sb.append(st)
            ident = cst.tile([128, 128], BF16)
            nc.sync.dma_start(out=ident, in_=ident_in.ap())
            for rt in range(2):
                at = acts.tile([128, D], F32, tag=f"a{rt}")
                nc.sync.dma_start(out=at, in_=a_in.ap()[128 * rt:128 * (rt + 1), :])
                a_sb.append(at)

            # ---------------- constants ----------------
            sel4 = cst.tile([4, 4, 256], BF16)
            nc.sync.dma_start(out=sel4, in_=sel_in.ap())
            vones = cst.tile([1, HP], BF16)
            nc.sync.dma_start(out=vones, in_=vones_in.ap())
            qb = cst.tile([128, 8], F32)
            nc.sync.dma_start(out=qb, in_=qb_in.ap().rearrange("t p -> p t"))
            eps_t = cst.tile([128, 1], F32)
            nc.vector.memset(eps_t, EPS)
            ones1 = cst.tile([1, 256], BF16)
            nc.vector.memset(ones1, 1.0)

            # ---------------- helpers ----------------
            def layernorm(dst, src):
                """dst[rt] = LN(src[rt]) without affine, bf16 out."""
                for rt in range(2):
                    stats = lnp.tile([128, 3, 6], F32, tag="lnstats")
                    mv = lnp.tile([128, 2], F32, tag="lnmv")
                    src3 = src[rt].rearrange("p (s c) -> p s c", s=3)
                    for sg in range(3):
                        nc.vector.bn_stats(out=stats[:, sg, :], in_=src3[:, sg, :])
                    nc.vector.bn_aggr(out=mv, in_=stats)
                    rstd = lnp.tile([128, 1], F32, tag="lnrstd")
                    nc.scalar.activation(out=rstd, in_=mv[:, 1:2], func=AF.Sqrt,
                                         bias=eps_t, scale=1.0)
                    nc.vector.reciprocal(out=rstd, in_=rstd)
                    nc.vector.tensor_scalar(out=dst[rt], in0=src[rt],
                                            scalar1=mv[:, 0:1], scalar2=rstd,
                                            op0=OP.subtract, op1=OP.mult)

            def transpose6(dst, src, tagp):
                """src: 2x[128,768] bf16 -> dst: 6x[128,256] bf16 (transposed)."""
                i = 0
                for rt in range(2):
                    for fc in range(FC):
                        pst = ps1.tile([128, 128], BF16, tag="ps1", name="pst")
                        nc.tensor.transpose(
                            out=pst, in_=src[rt][:, 128 * fc:128 * (fc + 1)],
                            identity=ident)
                        dslice = dst[fc][:, 128 * rt:128 * (rt + 1)]
                        if i % 2 == 0:
                            nc.vector.tensor_copy(out=dslice, in_=pst)
                        else:
                            nc.scalar.copy(out=dslice, in_=pst)
                        i += 1

            def proj_nat(lhsT, w_dram, n_fc, out_cols, bias_row=False, wtag="w768"):
                """Natural-orientation projection: returns 2 PSUM tiles [128,out_cols].

                lhsT: list of transposed-activation tiles [128, 256].
                Loops fc outer / row-tile inner so only a few weight chunks are
                alive at a time.
                """
                pss = [ps2.tile([128, out_cols], F32, tag="ps2", name="ps_nat") for _ in range(2)]
                ncol = [(c, min(c + 512, out_cols)) for c in range(0, out_cols, 512)]
                for fc in range(n_fc):
                    wt = wp.tile([128, out_cols], BF16, tag=wtag, name=f"wt_{wtag}")
                    nc.sync.dma_start(
                        out=wt, in_=w_dram.ap()[128 * fc:128 * (fc + 1), :])
                    for rt in range(2):
                        for (cs, ce) in ncol:
                            nc.tensor.matmul(
                                out=pss[rt][:, cs:ce],
                                lhsT=lhsT[fc][:, 128 * rt:128 * (rt + 1)],
                                rhs=wt[:, cs:ce],
                                start=(fc == 0),
                                stop=(fc == n_fc - 1 and not bias_row))
                if bias_row:
                    bt = wp.tile([1, out_cols], BF16, tag="wbias", name="wt_bias")
                    nc.sync.dma_start(out=bt, in_=w_dram.ap()[n_fc * 128:n_fc * 128 + 1, :])
                    for rt in range(2):
                        for (cs, ce) in ncol:
                            nc.tensor.matmul(
                                out=pss[rt][:, cs:ce],
                                lhsT=ones1[:, 128 * rt:128 * rt + 128],
                                rhs=bt[:, cs:ce],
                                start=False, stop=True)
                return pss

            # hoisted: s^T and both sigmoid output gates (independent of attention)
            sT = [acts.tile([128, 256], BF16, tag=f"sT{fc}", name=f"sT{fc}") for fc in range(FC)]
            transpose6(sT, s_sb, "sT")

            # ---------------- AdaLN 1 ----------------
            sn = [acts.tile([128, D], BF16, tag="lnout", bufs=4, name=f"sn{rt}") for rt in range(2)]
            an = [acts.tile([128, D], BF16, tag="lnout", bufs=4, name=f"an{rt}") for rt in range(2)]
            layernorm(sn, s_sb)
            layernorm(an, a_sb)

            snT = [acts.tile([128, 256], BF16, tag=f"snT{fc}", name=f"snT{fc}") for fc in range(FC)]
            transpose6(snT, sn, "snT")

            ps_ss1 = proj_nat(snT, w_ss1, FC, D, bias_row=True)
            sig1 = [acts.tile([128, D], BF16, tag=f"sig_{rt}", bufs=1, name=f"sig1_{rt}") for rt in range(2)]
            for rt in range(2):
                nc.scalar.activation(out=sig1[rt], in_=ps_ss1[rt], func=AF.Sigmoid)

            ps_sb1 = proj_nat(snT, w_sb1, FC, D)
            b_sb = [acts.tile([128, D], BF16, tag=f"ba2_{rt}", bufs=1, name=f"b{rt}") for rt in range(2)]
            for rt in range(2):
                tt = tmp.tile([128, D], BF16, tag="ttmp")
                nc.vector.tensor_mul(tt, an[rt], sig1[rt])
                nc.vector.tensor_add(b_sb[rt], tt, ps_sb1[rt])

            if stage == 1:
                for rt in range(2):
                    yt = tmp.tile([128, D], F32, tag="yt", bufs=1)
                    nc.vector.tensor_copy(out=yt, in_=b_sb[rt])
                    nc.sync.dma_start(out=y_out.ap()[128 * rt:128 * (rt + 1), :], in_=yt)
            if stage >= 2:
                bT = [acts.tile([128, 256], BF16, tag=f"bT{fc}", name=f"bT{fc}") for fc in range(FC)]
                transpose6(bT, b_sb, "bT")

                # ---------------- k^T, v (pre-collective) ----------------
                wk_sb = []
                for fc in range(FC):
                    wt = wcb.tile([128, HP], BF16, tag="wcb1024", bufs=7, name="wk")
                    nc.sync.dma_start(out=wt, in_=w_k.ap()[128 * fc:128 * (fc + 1), :])
                    wk_sb.append(wt)
                kt_sb = []
                for t in range(8):
                    ps = ps1.tile([128, 256], F32, tag="ps1", name="ps_cb")
                    for fc in range(FC):
                        nc.tensor.matmul(out=ps,
                                         lhsT=wk_sb[fc][:, 128 * t:128 * (t + 1)],
                                         rhs=bT[fc],
                                         start=(fc == 0), stop=(fc == FC - 1))
                    kt = acts.tile([128, 256], BF16, tag="ktl", bufs=3, name="ktl")
                    nc.scalar.copy(out=kt, in_=ps)
                    kt_sb.append(kt)

                wv_sb = []
                for fc in range(FC):
                    wt = wcb.tile([128, HP], BF16, tag="wcb1024", bufs=7, name="wv")
                    nc.sync.dma_start(out=wt, in_=w_v.ap()[128 * fc:128 * (fc + 1), :])
                    wv_sb.append(wt)
                v_sb = []
                for rt in range(2):
                    ps = ps2.tile([128, HP], F32, tag="ps2", name="ps_v")
                    for cs in (0, 512):
                        for fc in range(FC):
                            nc.tensor.matmul(
                                out=ps[:, cs:cs + 512],
                                lhsT=bT[fc][:, 128 * rt:128 * (rt + 1)],
                                rhs=wv_sb[fc][:, cs:cs + 512],
                                start=(fc == 0), stop=False)
                        nc.tensor.matmul(out=ps[:, cs:cs + 512],
                                         lhsT=ones1[:, :128],
                                         rhs=vones[:, cs:cs + 512],
                                         start=False, stop=True)
                    vt = acts.tile([128, HP], BF16, tag=f"v{rt}")
                    nc.scalar.copy(out=vt, in_=ps)
                    v_sb.append(vt)

                # ---------------- AllGather k^T/v within batch group ----------------
                kv_stage = dram.tile([SHARD], BF16)
                kv_gath = dram.tile([4 * SHARD], BF16)
                kst_k = kv_stage[0:KT_ELEMS].rearrange("(t p c) -> t p c", p=128, c=256)
                kst_v = kv_stage[KT_ELEMS:SHARD].rearrange("(j p c) -> j p c", p=128, c=HP)
                for t in range(8):
                    nc.gpsimd.dma_start(out=kst_k[t], in_=kt_sb[t])
                for rt in range(2):
                    nc.gpsimd.dma_start(out=kst_v[rt], in_=v_sb[rt])
                if os.environ.get("KSUB") != "noag":
                    nc.gpsimd.collective_compute(
                        "AllGather", OP.bypass,
                        replica_groups=[[0, 1, 2, 3], [4, 5, 6, 7]],
                        ins=[kv_stage.opt()],
                        outs=[kv_gath.opt()],
                    )
                kvg = kv_gath.rearrange("(r n) -> r n", n=SHARD)
                kt_view = kvg[:, 0:KT_ELEMS].rearrange(
                    "r (t p c) -> t p r c", p=128, c=256)
                v_view = kvg[:, KT_ELEMS:SHARD].rearrange(
                    "r (j p c) -> r j p c", p=128, c=HP)
                ktf = []
                for t in range(8):
                    kf = acts.tile([128, 4, 256], BF16, tag=f"ktf{t}")
                    nc.sync.dma_start(out=kf, in_=kt_view[t])
                    ktf.append(kf.rearrange("p r c -> p (r c)"))
                vf = []
                for kt in range(8):
                    vt = acts.tile([128, HP], BF16, tag=f"vf{kt}")
                    nc.sync.dma_start(out=vt, in_=v_view[kt // 2, kt % 2])
                    vf.append(vt)

                if stage == 2:
                    srcs = [ktf[0][:, 0:D], vf[0][:, 0:D]]
                    for rt in range(2):
                        yt = tmp.tile([128, D], F32, tag="yt", bufs=1)
                        nc.vector.tensor_copy(out=yt, in_=srcs[rt])
                        nc.sync.dma_start(out=y_out.ap()[128 * rt:128 * (rt + 1), :], in_=yt)
                if stage >= 3:
                    # sigmoid output gates (overlap the collective; only need sT)
                    ps_og = proj_nat(sT, w_op1, FC, D, bias_row=True)
                    og_sb = [acts.tile([128, D], BF16, tag=f"og{rt}", name=f"og{rt}") for rt in range(2)]
                    for rt in range(2):
                        nc.scalar.activation(out=og_sb[rt], in_=ps_og[rt], func=AF.Sigmoid)
                    ps_opg0 = proj_nat(sT, w_op2, FC, D, bias_row=True)
                    opg_sb = []
                    for rt in range(2):
                        opg = acts.tile([128, D], BF16, tag=f"opg{rt}", name=f"opg{rt}")
                        nc.scalar.activation(out=opg, in_=ps_opg0[rt], func=AF.Sigmoid)
                        opg_sb.append(opg)
                    # ---------------- q^T, gate^T (overlaps the collective) ----------
                    wq_sb = []
                    for fc in range(FC):
                        wt = wcb.tile([128, HP], BF16, tag="wcb1024", bufs=7, name="wq")
                        nc.sync.dma_start(out=wt, in_=w_q.ap()[128 * fc:128 * (fc + 1), :])
                        wq_sb.append(wt)
                    qt_sb = []
                    for t in range(8):
                        ps = ps1.tile([128, 256], F32, tag="ps1", name="ps_cb")
                        for fc in range(FC):
                            nc.tensor.matmul(out=ps,
                                             lhsT=wq_sb[fc][:, 128 * t:128 * (t + 1)],
                                             rhs=bT[fc],
                                             start=(fc == 0), stop=(fc == FC - 1))
                        qt = acts.tile([128, 256], BF16, tag=f"qt{t}")
                        nc.vector.tensor_scalar(out=qt, in0=ps, scalar1=qb[:, t:t + 1],
                                                scalar2=None, op0=OP.add)
                        qt_sb.append(qt)

                    wg_sb = []
                    for fc in range(FC):
                        wt = wcb.tile([128, HP], BF16, tag="wcb1024", bufs=7, name="wg")
                        nc.sync.dma_start(out=wt, in_=w_g.ap()[128 * fc:128 * (fc + 1), :])
                        wg_sb.append(wt)
                    gate_g = []
                    for t in range(8):
                        ps = ps1.tile([128, 256], F32, tag="ps1", name="ps_cb")
                        for fc in range(FC):
                            nc.tensor.matmul(out=ps,
                                             lhsT=wg_sb[fc][:, 128 * t:128 * (t + 1)],
                                             rhs=bT[fc],
                                             start=(fc == 0), stop=(fc == FC - 1))
                        gt = acts.tile([128, 256], BF16, tag=f"gt{t}")
                        nc.scalar.activation(out=gt, in_=ps, func=AF.Sigmoid)
                        gate_g.append(gt)


                    # AdaLN2 sn-side projections depend only on snT: run before attention
                    ps_ss2 = proj_nat(snT, w_ss2, FC, D, bias_row=True)
                    sig2 = [acts.tile([128, D], BF16, tag=f"sig_{rt}", bufs=1, name=f"sig2_{rt}") for rt in range(2)]
                    for rt in range(2):
                        nc.scalar.activation(out=sig2[rt], in_=ps_ss2[rt], func=AF.Sigmoid)
                    ps_sb2 = proj_nat(snT, w_sb2, FC, D)
                    sb2_sb = [acts.tile([128, D], BF16, tag=f"sb2_{rt}", name=f"sb2_{rt}") for rt in range(2)]
                    for rt in range(2):
                        nc.vector.tensor_copy(out=sb2_sb[rt], in_=ps_sb2[rt])

                    # ---------------- attention (grouped normalization, pipelined x) --
                    xT = [acts.tile([128, 256], BF16, tag=f"xT{t}", name=f"xT{t}") for t in range(8)]
                    S4 = [acts.tile([4, 256], F32, tag=f"S4_{g}", name=f"S4_{g}") for g in range(4)]
                    R4b = [acts.tile([4, 256], BF16, tag=f"R4b_{g}", name=f"R4b_{g}") for g in range(4)]
                    for t in range(8):  # head pairs
                        ps_pv = ps1.tile([128, 256], F32, tag="ps1", name="ps_pv")
                        for hb in range(2):
                            h = 2 * t + hb
                            base = 64 * hb
                            p_half = []
                            for half in range(2):
                                ez_t = ezp.tile([128, 4, 256], BF16, tag="ez")
                                nc.sync.dma_start(
                                    out=ez_t, in_=ez_in.ap()[h, :, 4 * half:4 * half + 4, :])
                                ps_s = ps2.tile([128, 1024], F32, tag="ps2", name="ps_s")
                                for k4 in range(4):
                                    kt = 4 * half + k4
                                    nc.tensor.matmul(
                                        out=ps_s[:, 256 * k4:256 * (k4 + 1)],
                                        lhsT=ktf[t][base:base + 48, 128 * kt:128 * (kt + 1)],
                                        rhs=qt_sb[t][base:base + 48, :],
                                        start=True, stop=True)
                                p = pp.tile([128, 1024], BF16, tag="p")
                                nc.scalar.activation(out=p, in_=ps_s, func=AF.Exp)
                                nc.vector.tensor_mul(p, p, ez_t.rearrange("p a b -> p (a b)"))
                                p_half.append(p)
                            for kt in range(8):
                                nc.tensor.matmul(
                                    out=ps_pv[base:base + 64, :],
                                    lhsT=vf[kt][:, HDP * h:HDP * (h + 1)],
                                    rhs=p_half[kt // 4][:, 256 * (kt % 4):256 * (kt % 4 + 1)],
                                    start=(h % 2 == 0 and kt == 0),
                                    stop=(h % 2 == 1 and kt == 7),
                                    tile_position=(0, base) if hb else None)
                        # row sums sit at partitions 0 / 64 (ones column of padded V)
                        g, pq = t // 2, t % 2
                        tsum = tmp.tile([128, 256], F32, tag="tsum")
                        nc.vector.tensor_copy(out=tsum[0:1, :], in_=ps_pv[0:1, :])
                        nc.vector.tensor_copy(out=tsum[64:65, :], in_=ps_pv[64:65, :])
                        nc.sync.dma_start(out=S4[g][2 * pq:2 * pq + 1, :], in_=tsum[0:1, :])
                        nc.sync.dma_start(out=S4[g][2 * pq + 1:2 * pq + 2, :], in_=tsum[64:65, :])
                        nc.vector.tensor_mul(xT[t], ps_pv, gate_g[t])
                        if pq == 1:  # group of two pairs done: normalize early
                            nc.vector.reciprocal_approx_fast(out=S4[g], in_=S4[g])
                            nc.vector.tensor_copy(out=R4b[g], in_=S4[g])
                            for tq in (t - 1, t):
                                ps_bc = ps1.tile([128, 256], F32, tag="ps1", name="ps_bc")
                                nc.tensor.matmul(
                                    out=ps_bc, lhsT=sel4[:, g, 128 * (tq % 2):128 * (tq % 2) + 128],
                                    rhs=R4b[g], start=True, stop=True)
                                nc.vector.tensor_mul(xT[tq], xT[tq], ps_bc)

                    # output projection: x = xT.T @ o_w (starts as soon as slots free)
                    ps_x = [ps2.tile([128, D], F32, tag="ps2", name="ps_x") for _ in range(2)]
                    for tq in range(8):
                        wt_ow = wp.tile([128, D], BF16, tag="w768", name="wt_ow")
                        nc.sync.dma_start(out=wt_ow, in_=w_ow.ap()[128 * tq:128 * (tq + 1), :])
                        for rt in range(2):
                            for cs in (0, 512):
                                ce = min(cs + 512, D)
                                nc.tensor.matmul(
                                    out=ps_x[rt][:, cs:ce],
                                    lhsT=xT[tq][:, 128 * rt:128 * (rt + 1)],
                                    rhs=wt_ow[:, cs:ce],
                                    start=(tq == 0), stop=(tq == 7))

                    a1_sb = []
                    for rt in range(2):
                        xg = tmp.tile([128, D], BF16, tag="xg")
                        nc.vector.tensor_mul(xg, ps_x[rt], og_sb[rt])
                        a1 = acts.tile([128, D], F32, tag=f"a1_{rt}")
                        nc.vector.tensor_add(a1, a_sb[rt], xg)
                        a1_sb.append(a1)

                    if stage == 3:
                        for rt in range(2):
                            nc.sync.dma_start(out=y_out.ap()[128 * rt:128 * (rt + 1), :], in_=a1_sb[rt])
                    if stage >= 4:
                        # ---------------- AdaLN 2 (sn reused: snw folded on host) --------
                        an2 = [acts.tile([128, D], BF16, tag="lnout", bufs=4, name=f"an2_{rt}") for rt in range(2)]
                        layernorm(an2, a1_sb)
                        a2_sb = [acts.tile([128, D], BF16, tag=f"ba2_{rt}", bufs=1, name=f"a2_{rt}") for rt in range(2)]
                        for rt in range(2):
                            tt = tmp.tile([128, D], BF16, tag="ttmp")
                            nc.vector.tensor_mul(tt, an2[rt], sig2[rt])
                            nc.vector.tensor_add(a2_sb[rt], tt, sb2_sb[rt])
                        a2T = [acts.tile([128, 256], BF16, tag=f"a2T{fc}", name=f"a2T{fc}") for fc in range(FC)]
                        transpose6(a2T, a2_sb, "a2T")

                        # ---------------- transition (feature-on-partition) --------------
                        def proj_convB(w_dram, rhs_tiles, n_oct, wtagbase):
                            # half-width weight chunks: octs 0-5 use half 0, octs 6-11 half 1,
                            # so half-0 slots recycle to the next projection 6 octs earlier.
                            wts = [[None] * FC for _ in range(2)]
                            def load_half(hh):
                                for fc in range(FC):
                                    wt = wcb.tile([128, 768], BF16, tag="wcb768t", bufs=14,
                                                  name=wtagbase)
                                    nc.sync.dma_start(
                                        out=wt,
                                        in_=w_dram.ap()[128 * fc:128 * (fc + 1),
                                                        768 * hh:768 * (hh + 1)])
                                    wts[hh][fc] = wt
                            load_half(0)
                            load_half(1)
                            outs = []
                            for t in range(n_oct):
                                hh, tt = t // 6, t % 6
                                ps = ps1.tile([128, 256], F32, tag="ps1", name="ps_cb")
                                for fc in range(FC):
                                    nc.tensor.matmul(out=ps,
                                                     lhsT=wts[hh][fc][:, 128 * tt:128 * (tt + 1)],
                                                     rhs=rhs_tiles[fc],
                                                     start=(fc == 0), stop=(fc == FC - 1))
                                outs.append(ps)
                            return outs

                        # (op gate hoisted to kernel start)
                        hT = [acts.tile([128, 256], BF16, tag=f"hT{t}", name=f"hT{t}") for t in range(12)]
                        u_sb = []
                        for t, ps in enumerate(proj_convB(w_swu, a2T, 12, "wsu")):
                            ut = acts.tile([128, 256], BF16, tag=f"u{t}", name=f"u{t}")
                            nc.vector.tensor_copy(out=ut, in_=ps)
                            u_sb.append(ut)
                        sg_sb = []
                        for t, ps in enumerate(proj_convB(w_swg, a2T, 12, "wsg")):
                            st_ = acts.tile([128, 256], BF16, tag=f"sg{t}", name=f"sg{t}")
                            nc.scalar.activation(out=st_, in_=ps, func=AF.Silu)
                            sg_sb.append(st_)
                        ps_t = [ps2.tile([128, D], F32, tag="ps2", name="ps_t") for _ in range(2)]
                        for t, ps in enumerate(proj_convB(w_ab, a2T, 12, "wab")):
                            hu = tmp.tile([128, 256], BF16, tag="hu")
                            nc.vector.tensor_mul(hu, sg_sb[t], u_sb[t])
                            nc.vector.tensor_mul(hT[t], hu, ps)
                            wt_ba = wp.tile([128, D], BF16, tag="w768", name="wt_ba")
                            nc.sync.dma_start(out=wt_ba, in_=w_ba.ap()[128 * t:128 * (t + 1), :])
                            for rt in range(2):
                                for cs in (0, 512):
                                    ce = min(cs + 512, D)
                                    nc.tensor.matmul(
                                        out=ps_t[rt][:, cs:ce],
                                        lhsT=hT[t][:, 128 * rt:128 * (rt + 1)],
                                        rhs=wt_ba[:, cs:ce],
                                        start=(t == 0), stop=(t == 11))

                        for rt in range(2):
                            yt = tmp.tile([128, D], F32, tag="yt", bufs=1)
                            for (hs, he) in ((0, 384), (384, D)):
                                tg = tmp.tile([128, 384], BF16, tag="tg")
                                nc.vector.tensor_mul(
                                    tg, ps_t[rt][:, hs:he], opg_sb[rt][:, hs:he])
                                nc.vector.tensor_add(
                                    yt[:, hs:he], a1_sb[rt][:, hs:he], tg)
                                nc.sync.dma_start(
                                    out=y_out.ap()[128 * rt:128 * (rt + 1), hs:he],
                                    in_=yt[:, hs:he])

    nc.finalize()
    return nc


def _get_nc():
    if "nc" not in _NC_CACHE:
        _NC_CACHE["nc"] = _build_nc()
    return _NC_CACHE["nc"]


def _pad_cols(w):
    """[768, 768] -> [768, 1024]: each head's 48 cols at a 64-aligned block."""
    wp = np.zeros((D, HP), np.float32)
    wp.reshape(D, H, HDP)[:, :, :HD] = np.asarray(w, np.float32).reshape(D, H, HD)
    return wp


def _bf(x):
    return np.ascontiguousarray(np.asarray(x, np.float32).astype(NPBF16))


def kernel(**inputs):
    a = np.asarray(inputs["a"], np.float32)
    s = np.asarray(inputs["s"], np.float32)
    z = np.asarray(inputs["z"], np.float32)

    snw1 = np.asarray(inputs["adaln1_snw"], np.float32)[:, None]
    snw2 = np.asarray(inputs["adaln2_snw"], np.float32)[:, None]
    w_ss1 = _bf(np.vstack([snw1 * np.asarray(inputs["adaln1_ssw"], np.float32),
                           np.asarray(inputs["adaln1_ssb"], np.float32)[None]]))
    w_sb1 = _bf(snw1 * np.asarray(inputs["adaln1_sbw"], np.float32))
    w_ss2 = _bf(np.vstack([snw2 * np.asarray(inputs["adaln2_ssw"], np.float32),
                           np.asarray(inputs["adaln2_ssb"], np.float32)[None]]))
    w_sb2 = _bf(snw2 * np.asarray(inputs["adaln2_sbw"], np.float32))

    w_q = _bf(_pad_cols(inputs["q_w"]) * SCALE)
    qb_p = np.zeros((H, HDP), np.float32)
    qb_p[:, :HD] = np.asarray(inputs["q_b"], np.float32).reshape(H, HD) * SCALE
    qb_p = np.ascontiguousarray(qb_p.reshape(8, 128))
    w_k = _bf(_pad_cols(inputs["k_w"]))
    w_g = _bf(_pad_cols(inputs["g_w"]))
    w_vp = np.zeros((D, HP), np.float32)
    w_vp.reshape(D, H, HDP)[:, :, 1:HD + 1] = \
        np.asarray(inputs["v_w"], np.float32).reshape(D, H, HD)
    w_v = _bf(w_vp)
    w_ow = np.zeros((HP, D), np.float32)
    w_ow.reshape(H, HDP, D)[:, 1:HD + 1, :] = \
        np.asarray(inputs["o_w"], np.float32).reshape(H, HD, D)
    w_ow = _bf(w_ow)
    w_op1 = _bf(np.vstack([np.asarray(inputs["outproj_w"], np.float32),
                           np.asarray(inputs["outproj_b"], np.float32)[None]]))
    w_op2 = _bf(np.vstack([np.asarray(inputs["op_w"], np.float32),
                           np.asarray(inputs["op_b"], np.float32)[None]]))
    sw = np.asarray(inputs["swish_w"], np.float32)
    w_swu = _bf(sw[:, :HID])
    w_swg = _bf(sw[:, HID:])
    w_ab = _bf(inputs["a2b_w"])
    w_ba = _bf(inputs["b2a_w"])

    ident = _bf(np.eye(128))
    sel = np.zeros((4, 4, 2, 128), np.float32)
    for g in range(4):
        for r in range(4):
            for p in range(2):
                for m in range(128):
                    if r == 2 * p + m // 64:
                        sel[g, r, p, m] = 1.0
    sel = _bf(sel.reshape(4, 4, 256).transpose(1, 0, 2))
    vones = np.zeros((1, HP), np.float32)
    vones.reshape(H, HDP)[:, 0] = 1.0
    vones = _bf(vones)

    shared = dict(
        ident_in=ident, sel_in=sel, vones_in=vones, qb_in=qb_p,
        w_ss1=w_ss1, w_sb1=w_sb1, w_ss2=w_ss2, w_sb2=w_sb2,
        w_q=w_q, w_k=w_k, w_g=w_g, w_v=w_v, w_ow=w_ow,
        w_op1=w_op1, w_op2=w_op2, w_swu=w_swu, w_swg=w_swg,
        w_ab=w_ab, w_ba=w_ba,
    )

    in_maps = []
    for c in range(8):
        beta, q0 = c // 4, 256 * (c % 4)
        rows = slice(q0, q0 + 256)
        ez = np.exp(z[:, beta, rows, :])          # [16, 256, 1024]
        ez = ez.transpose(0, 2, 1)                # [16, 1024k, 256q]
        ez = ez.reshape(H, 8, 128, R).transpose(0, 2, 1, 3)  # [16,128,8,256]
        m = dict(shared)
        m["a_in"] = np.ascontiguousarray(a[beta, rows, :])
        m["s_in"] = _bf(s[beta, rows, :])
        m["ez_in"] = np.ascontiguousarray(ez.astype(NPBF16))
        in_maps.append(m)

    nc = _get_nc()
    global _LAST_IN_MAPS
    _LAST_IN_MAPS = in_maps
    res = run_bass_kernel_spmd(nc, in_maps, core_ids=list(range(8)))

    out = np.empty((B, N, D), np.float32)
    for c in range(8):
        beta, q0 = c // 4, 256 * (c % 4)
        out[beta, q0:q0 + 256, :] = res.results[c]["y"]
    return out



# revision 4
# speedup vs baseline: 7.8620x; 7.8620x over previous
"""DiffusionTransformerLayer on 8 Trainium2 NeuronCores.

Sharding: rows (B*N = 2048 tokens) split 256/core; attention K/V all-gathered
within each batch's 4-core group (one 4-rank AllGather of bf16 k^T/v).

Layout strategy:
  - "natural" activations: [rows(part), feat(free)]; LN / gates / residuals here.
  - matmul inputs are transposed on the PE (128x128 transposes) since the
    tensor engine contracts over the partition axis.
  - attention runs feature-on-partition: qT/kT produced directly by projecting
    with host-padded weights (heads padded 48->64 so each head sits at
    partition base 0/64 of an SBUF tile). Scores are computed transposed
    (ST[k,q]), softmax over k becomes: exp on ACT, pair-bias via elementwise
    multiply with host-precomputed exp(z^T), and the row-sum arrives free as a
    ones-column in the padded V weights through the P@V matmul.
  - all matmul operands bf16 (fp32 PSUM accumulation); residual stream fp32.
"""
import os

import numpy as np
import ml_dtypes

import concourse.bacc as bacc
import concourse.bass as bass
import concourse.tile as tile
from concourse import mybir
from concourse.bass_utils import run_bass_kernel_spmd

F32 = mybir.dt.float32
BF16 = mybir.dt.bfloat16
AF = mybir.ActivationFunctionType
OP = mybir.AluOpType
NPBF16 = ml_dtypes.bfloat16

B, N, D, H = 2, 1024, 768, 16
HD, HDP = 48, 64
HP = H * HDP          # 1024 padded head dims
HID = 1536
R = 256               # rows per core
FC = D // 128         # 6 feature chunks
EPS = 1e-5
SCALE = HD ** -0.5
KT_ELEMS = HP * R     # kT shard elems
V_ELEMS = R * HP      # v shard elems
SHARD = KT_ELEMS + V_ELEMS

_NC_CACHE = {}

# Single packed input blob (bf16 elements; fp32 regions stored as raw bytes
# and bitcast on device). (name, shape, is_f32). Order = blob layout.
_LAYOUT = [
    ("a", (R, D), True),
    ("qb", (8, 128), True),
    ("s", (R, D), False),
    ("ez", (H, 128, 8, R), False),
    ("ident", (128, 128), False),
    ("sel", (4, 4, 256), False),
    ("vones", (1, HP), False),
    ("w_ss1", (D + 1, D), False),
    ("w_sb1", (D, D), False),
    ("w_ss2", (D + 1, D), False),
    ("w_sb2", (D, D), False),
    ("w_q", (D, HP), False),
    ("w_k", (D, HP), False),
    ("w_g", (D, HP), False),
    ("w_v", (D, HP), False),
    ("w_ow", (HP, D), False),
    ("w_op1", (D + 1, D), False),
    ("w_op2", (D + 1, D), False),
    ("w_swu", (D, HID), False),
    ("w_swg", (D, HID), False),
    ("w_ab", (D, HID), False),
    ("w_ba", (HID, D), False),
]


def _layout_offsets():
    offs, off = {}, 0
    for name, shape, is_f32 in _LAYOUT:
        n = int(np.prod(shape)) * (2 if is_f32 else 1)
        offs[name] = (off, n, shape, is_f32)
        off += n
    return offs, off


_OFFS, _BLOB_LEN = _layout_offsets()


def _build_nc():
    stage = int(os.environ.get("KSTAGE", "4"))
    nc = bacc.Bacc("TRN2", target_bir_lowering=False, debug=False, num_devices=8)

    blob = nc.dram_tensor("blob", [_BLOB_LEN], BF16, kind="ExternalInput")

    def vin(name):
        off, n, shape, is_f32 = _OFFS[name]
        ap = blob.ap()[off:off + n]
        if is_f32:
            ap = ap.bitcast(F32)
        dims = " ".join(f"d{i}" for i in range(len(shape)))
        kw = {f"d{i}": shape[i] for i in range(1, len(shape))}
        return ap.rearrange(f"({dims}) -> {dims}", **kw)

    a_in = vin("a")
    s_in = vin("s")
    ez_in = vin("ez")
    ident_in = vin("ident")
    sel_in = vin("sel")
    vones_in = vin("vones")
    qb_in = vin("qb")
    w_ss1 = vin("w_ss1")
    w_sb1 = vin("w_sb1")
    w_ss2 = vin("w_ss2")
    w_sb2 = vin("w_sb2")
    w_q = vin("w_q")
    w_k = vin("w_k")
    w_g = vin("w_g")
    w_v = vin("w_v")
    w_ow = vin("w_ow")
    w_op1 = vin("w_op1")
    w_op2 = vin("w_op2")
    w_swu = vin("w_swu")
    w_swg = vin("w_swg")
    w_ab = vin("w_ab")
    w_ba = vin("w_ba")

    y_out = nc.dram_tensor("y", [R, D], F32, kind="ExternalOutput")

    with tile.TileContext(nc) as tc:
        from contextlib import ExitStack
        with ExitStack() as ctx:
            cst = ctx.enter_context(tc.tile_pool(name="cst", bufs=1))
            acts = ctx.enter_context(tc.tile_pool(name="acts", bufs=1))
            lnp = ctx.enter_context(tc.tile_pool(name="lnp", bufs=3))
            wp = ctx.enter_context(tc.tile_pool(name="wp", bufs=4))
            wcb = ctx.enter_context(tc.tile_pool(name="wcb", bufs=1))
            ezp = ctx.enter_context(tc.tile_pool(name="ezp", bufs=4))
            pp = ctx.enter_context(tc.tile_pool(name="pp", bufs=5))
            tmp = ctx.enter_context(tc.tile_pool(name="tmp", bufs=2))
            dram = ctx.enter_context(tc.tile_pool(name="dram", bufs=1, space="DRAM"))
            ps2 = ctx.enter_context(tc.tile_pool(name="ps2", bufs=2, space="PSUM"))
            ps1 = ctx.enter_context(tc.tile_pool(name="ps1", bufs=4, space="PSUM"))

            # ---------------- load s (+ident) first: feeds the first PE work --
            a_sb, s_sb = [], []
            for rt in range(2):
                st = acts.tile([128, D], BF16, tag=f"s{rt}")
                nc.sync.dma_start(out=st, in_=s_in[128 * rt:128 * (rt + 1), :])
                s_sb.append(st)
            ident = cst.tile([128, 128], BF16)
            nc.sync.dma_start(out=ident, in_=ident_in)
            for rt in range(2):
                at = acts.tile([128, D], F32, tag=f"a{rt}")
                nc.sync.dma_start(out=at, in_=a_in[128 * rt:128 * (rt + 1), :])
                a_sb.append(at)

            # ---------------- constants ----------------
            sel4 = cst.tile([4, 4, 256], BF16)
            nc.sync.dma_start(out=sel4, in_=sel_in)
            vones = cst.tile([1, HP], BF16)
            nc.sync.dma_start(out=vones, in_=vones_in)
            qb = cst.tile([128, 8], F32)
            nc.sync.dma_start(out=qb, in_=qb_in.rearrange("t p -> p t"))
            eps_t = cst.tile([128, 1], F32)
            nc.vector.memset(eps_t, EPS)
            ones1 = cst.tile([1, 256], BF16)
            nc.vector.memset(ones1, 1.0)

            # ---------------- helpers ----------------
            def layernorm(dst, src):
                """dst[rt] = LN(src[rt]) without affine, bf16 out."""
                for rt in range(2):
                    stats = lnp.tile([128, 3, 6], F32, tag="lnstats")
                    mv = lnp.tile([128, 2], F32, tag="lnmv")
                    src3 = src[rt].rearrange("p (s c) -> p s c", s=3)
                    for sg in range(3):
                        nc.vector.bn_stats(out=stats[:, sg, :], in_=src3[:, sg, :])
                    nc.vector.bn_aggr(out=mv, in_=stats)
                    rstd = lnp.tile([128, 1], F32, tag="lnrstd")
                    nc.scalar.activation(out=rstd, in_=mv[:, 1:2], func=AF.Sqrt,
                                         bias=eps_t, scale=1.0)
                    nc.vector.reciprocal(out=rstd, in_=rstd)
                    nc.vector.tensor_scalar(out=dst[rt], in0=src[rt],
                                            scalar1=mv[:, 0:1], scalar2=rstd,
                                            op0=OP.subtract, op1=OP.mult)

            def transpose6(dst, src, tagp):
                """src: 2x[128,768] bf16 -> dst: 6x[128,256] bf16 (transposed)."""
                i = 0
                for rt in range(2):
                    for fc in range(FC):
                        pst = ps1.tile([128, 128], BF16, tag="ps1", name="pst")
                        nc.tensor.transpose(
                            out=pst, in_=src[rt][:, 128 * fc:128 * (fc + 1)],
                            identity=ident)
                        dslice = dst[fc][:, 128 * rt:128 * (rt + 1)]
                        if i % 2 == 0:
                            nc.vector.tensor_copy(out=dslice, in_=pst)
                        else:
                            nc.scalar.copy(out=dslice, in_=pst)
                        i += 1

            def proj_nat(lhsT, w_dram, n_fc, out_cols, bias_row=False, wtag="w768"):
                """Natural-orientation projection: returns 2 PSUM tiles [128,out_cols].

                lhsT: list of transposed-activation tiles [128, 256].
                Loops fc outer / row-tile inner so only a few weight chunks are
                alive at a time.
                """
                pss = [ps2.tile([128, out_cols], F32, tag="ps2", name="ps_nat") for _ in range(2)]
                ncol = [(c, min(c + 512, out_cols)) for c in range(0, out_cols, 512)]
                for fc in range(n_fc):
                    wt = wp.tile([128, out_cols], BF16, tag=wtag, name=f"wt_{wtag}")
                    nc.sync.dma_start(
                        out=wt, in_=w_dram[128 * fc:128 * (fc + 1), :])
                    for rt in range(2):
                        for (cs, ce) in ncol:
                            nc.tensor.matmul(
                                out=pss[rt][:, cs:ce],
                                lhsT=lhsT[fc][:, 128 * rt:128 * (rt + 1)],
                                rhs=wt[:, cs:ce],
                                start=(fc == 0),
                                stop=(fc == n_fc - 1 and not bias_row))
                if bias_row:
                    bt = wp.tile([1, out_cols], BF16, tag="wbias", name="wt_bias")
                    nc.sync.dma_start(out=bt, in_=w_dram[n_fc * 128:n_fc * 128 + 1, :])
                    for rt in range(2):
                        for (cs, ce) in ncol:
                            nc.tensor.matmul(
                                out=pss[rt][:, cs:ce],
                                lhsT=ones1[:, 128 * rt:128 * rt + 128],
                                rhs=bt[:, cs:ce],
                                start=False, stop=True)
                return pss

            # hoisted: s^T and both sigmoid output gates (independent of attention)
            sT = [acts.tile([128, 256], BF16, tag=f"sT{fc}", name=f"sT{fc}") for fc in range(FC)]
            transpose6(sT, s_sb, "sT")

            # ---------------- AdaLN 1 ----------------
            sn = [acts.tile([128, D], BF16, tag="lnout", bufs=4, name=f"sn{rt}") for rt in range(2)]
            an = [acts.tile([128, D], BF16, tag="lnout", bufs=4, name=f"an{rt}") for rt in range(2)]
            layernorm(sn, s_sb)
            layernorm(an, a_sb)

            snT = [acts.tile([128, 256], BF16, tag=f"snT{fc}", name=f"snT{fc}") for fc in range(FC)]
            transpose6(snT, sn, "snT")

            ps_ss1 = proj_nat(snT, w_ss1, FC, D, bias_row=True)
            sig1 = [acts.tile([128, D], BF16, tag=f"sig_{rt}", bufs=1, name=f"sig1_{rt}") for rt in range(2)]
            for rt in range(2):
                nc.scalar.activation(out=sig1[rt], in_=ps_ss1[rt], func=AF.Sigmoid)

            ps_sb1 = proj_nat(snT, w_sb1, FC, D)
            b_sb = [acts.tile([128, D], BF16, tag=f"ba2_{rt}", bufs=1, name=f"b{rt}") for rt in range(2)]
            for rt in range(2):
                tt = tmp.tile([128, D], BF16, tag="ttmp")
                nc.vector.tensor_mul(tt, an[rt], sig1[rt])
                nc.vector.tensor_add(b_sb[rt], tt, ps_sb1[rt])

            if stage == 1:
                for rt in range(2):
                    yt = tmp.tile([128, D], F32, tag="yt", bufs=1)
                    nc.vector.tensor_copy(out=yt, in_=b_sb[rt])
                    nc.sync.dma_start(out=y_out.ap()[128 * rt:128 * (rt + 1), :], in_=yt)
            if stage >= 2:
                bT = [acts.tile([128, 256], BF16, tag=f"bT{fc}", name=f"bT{fc}") for fc in range(FC)]
                transpose6(bT, b_sb, "bT")

                # ---------------- k^T, v (pre-collective) ----------------
                wk_sb = []
                for fc in range(FC):
                    wt = wcb.tile([128, HP], BF16, tag="wcb1024", bufs=7, name="wk")
                    nc.sync.dma_start(out=wt, in_=w_k[128 * fc:128 * (fc + 1), :])
                    wk_sb.append(wt)
                kt_sb = []
                for t in range(8):
                    ps = ps1.tile([128, 256], F32, tag="ps1", name="ps_cb")
                    for fc in range(FC):
                        nc.tensor.matmul(out=ps,
                                         lhsT=wk_sb[fc][:, 128 * t:128 * (t + 1)],
                                         rhs=bT[fc],
                                         start=(fc == 0), stop=(fc == FC - 1))
                    kt = acts.tile([128, 256], BF16, tag="ktl", bufs=3, name="ktl")
                    nc.scalar.copy(out=kt, in_=ps)
                    kt_sb.append(kt)

                wv_sb = []
                for fc in range(FC):
                    wt = wcb.tile([128, HP], BF16, tag="wcb1024", bufs=7, name="wv")
                    nc.sync.dma_start(out=wt, in_=w_v[128 * fc:128 * (fc + 1), :])
                    wv_sb.append(wt)
                v_sb = []
                for rt in range(2):
                    ps = ps2.tile([128, HP], F32, tag="ps2", name="ps_v")
                    for cs in (0, 512):
                        for fc in range(FC):
                            nc.tensor.matmul(
                                out=ps[:, cs:cs + 512],
                                lhsT=bT[fc][:, 128 * rt:128 * (rt + 1)],
                                rhs=wv_sb[fc][:, cs:cs + 512],
                                start=(fc == 0), stop=False)
                        nc.tensor.matmul(out=ps[:, cs:cs + 512],
                                         lhsT=ones1[:, :128],
                                         rhs=vones[:, cs:cs + 512],
                                         start=False, stop=True)
                    vt = acts.tile([128, HP], BF16, tag=f"v{rt}")
                    nc.scalar.copy(out=vt, in_=ps)
                    v_sb.append(vt)

                # ---------------- AllGather k^T/v within batch group ----------------
                kv_stage = dram.tile([SHARD], BF16)
                kv_gath = dram.tile([4 * SHARD], BF16)
                kst_k = kv_stage[0:KT_ELEMS].rearrange("(t p c) -> t p c", p=128, c=256)
                kst_v = kv_stage[KT_ELEMS:SHARD].rearrange("(j p c) -> j p c", p=128, c=HP)
                for t in range(8):
                    nc.gpsimd.dma_start(out=kst_k[t], in_=kt_sb[t])
                for rt in range(2):
                    nc.gpsimd.dma_start(out=kst_v[rt], in_=v_sb[rt])
                if os.environ.get("KSUB") != "noag":
                    nc.gpsimd.collective_compute(
                        "AllGather", OP.bypass,
                        replica_groups=[[0, 1, 2, 3], [4, 5, 6, 7]],
                        ins=[kv_stage.opt()],
                        outs=[kv_gath.opt()],
                    )
                kvg = kv_gath.rearrange("(r n) -> r n", n=SHARD)
                kt_view = kvg[:, 0:KT_ELEMS].rearrange(
                    "r (t p c) -> t p r c", p=128, c=256)
                v_view = kvg[:, KT_ELEMS:SHARD].rearrange(
                    "r (j p c) -> r j p c", p=128, c=HP)
                ktf = []
                for t in range(8):
                    kf = acts.tile([128, 4, 256], BF16, tag=f"ktf{t}")
                    nc.sync.dma_start(out=kf, in_=kt_view[t])
                    ktf.append(kf.rearrange("p r c -> p (r c)"))
                vf = []
                for kt in range(8):
                    vt = acts.tile([128, HP], BF16, tag=f"vf{kt}")
                    nc.sync.dma_start(out=vt, in_=v_view[kt // 2, kt % 2])
                    vf.append(vt)

                if stage == 2:
                    srcs = [ktf[0][:, 0:D], vf[0][:, 0:D]]
                    for rt in range(2):
                        yt = tmp.tile([128, D], F32, tag="yt", bufs=1)
                        nc.vector.tensor_copy(out=yt, in_=srcs[rt])
                        nc.sync.dma_start(out=y_out.ap()[128 * rt:128 * (rt + 1), :], in_=yt)
                if stage >= 3:
                    # sigmoid output gates (overlap the collective; only need sT)
                    ps_og = proj_nat(sT, w_op1, FC, D, bias_row=True)
                    og_sb = [acts.tile([128, D], BF16, tag=f"og{rt}", name=f"og{rt}") for rt in range(2)]
                    for rt in range(2):
                        nc.scalar.activation(out=og_sb[rt], in_=ps_og[rt], func=AF.Sigmoid)
                    ps_opg0 = proj_nat(sT, w_op2, FC, D, bias_row=True)
                    opg_sb = []
                    for rt in range(2):
                        opg = acts.tile([128, D], BF16, tag=f"opg{rt}", name=f"opg{rt}")
                        nc.scalar.activation(out=opg, in_=ps_opg0[rt], func=AF.Sigmoid)
                        opg_sb.append(opg)
                    # ---------------- q^T, gate^T (overlaps the collective) ----------
                    wq_sb = []
                    for fc in range(FC):
                        wt = wcb.tile([128, HP], BF16, tag="wcb1024", bufs=7, name="wq")
                        nc.sync.dma_start(out=wt, in_=w_q[128 * fc:128 * (fc + 1), :])
                        wq_sb.append(wt)
                    qt_sb = []
                    for t in range(8):
                        ps = ps1.tile([128, 256], F32, tag="ps1", name="ps_cb")
                        for fc in range(FC):
                            nc.tensor.matmul(out=ps,
                                             lhsT=wq_sb[fc][:, 128 * t:128 * (t + 1)],
                                             rhs=bT[fc],
                                             start=(fc == 0), stop=(fc == FC - 1))
                        qt = acts.tile([128, 256], BF16, tag=f"qt{t}")
                        nc.vector.tensor_scalar(out=qt, in0=ps, scalar1=qb[:, t:t + 1],
                                                scalar2=None, op0=OP.add)
                        qt_sb.append(qt)

                    wg_sb = []
                    for fc in range(FC):
                        wt = wcb.tile([128, HP], BF16, tag="wcb1024", bufs=7, name="wg")
                        nc.sync.dma_start(out=wt, in_=w_g[128 * fc:128 * (fc + 1), :])
                        wg_sb.append(wt)
                    gate_g = []
                    for t in range(8):
                        ps = ps1.tile([128, 256], F32, tag="ps1", name="ps_cb")
                        for fc in range(FC):
                            nc.tensor.matmul(out=ps,
                                             lhsT=wg_sb[fc][:, 128 * t:128 * (t + 1)],
                                             rhs=bT[fc],
                                             start=(fc == 0), stop=(fc == FC - 1))
                        gt = acts.tile([128, 256], BF16, tag=f"gt{t}")
                        nc.scalar.activation(out=gt, in_=ps, func=AF.Sigmoid)
                        gate_g.append(gt)


                    # AdaLN2 sn-side projections depend only on snT: run before attention
                    ps_ss2 = proj_nat(snT, w_ss2, FC, D, bias_row=True)
                    sig2 = [acts.tile([128, D], BF16, tag=f"sig_{rt}", bufs=1, name=f"sig2_{rt}") for rt in range(2)]
                    for rt in range(2):
                        nc.scalar.activation(out=sig2[rt], in_=ps_ss2[rt], func=AF.Sigmoid)
                    ps_sb2 = proj_nat(snT, w_sb2, FC, D)
                    sb2_sb = [acts.tile([128, D], BF16, tag=f"sb2_{rt}", name=f"sb2_{rt}") for rt in range(2)]
                    for rt in range(2):
                        nc.vector.tensor_copy(out=sb2_sb[rt], in_=ps_sb2[rt])

                    # ---------------- attention (grouped normalization, pipelined x) --
                    xT = [acts.tile([128, 256], BF16, tag=f"xT{t}", name=f"xT{t}") for t in range(8)]
                    S4 = [acts.tile([4, 256], F32, tag=f"S4_{g}", name=f"S4_{g}") for g in range(4)]
                    R4b = [acts.tile([4, 256], BF16, tag=f"R4b_{g}", name=f"R4b_{g}") for g in range(4)]
                    for t in range(8):  # head pairs
                        ps_pv = ps1.tile([128, 256], F32, tag="ps1", name="ps_pv")
                        for hb in range(2):
                            h = 2 * t + hb
                            base = 64 * hb
                            p_half = []
                            for half in range(2):
                                ez_t = ezp.tile([128, 4, 256], BF16, tag="ez")
                                nc.sync.dma_start(
                                    out=ez_t, in_=ez_in[h, :, 4 * half:4 * half + 4, :])
                                ps_s = ps2.tile([128, 1024], F32, tag="ps2", name="ps_s")
                                for k4 in range(4):
                                    kt = 4 * half + k4
                                    nc.tensor.matmul(
                                        out=ps_s[:, 256 * k4:256 * (k4 + 1)],
                                        lhsT=ktf[t][base:base + 48, 128 * kt:128 * (kt + 1)],
                                        rhs=qt_sb[t][base:base + 48, :],
                                        start=True, stop=True)
                                p = pp.tile([128, 1024], BF16, tag="p")
                                nc.scalar.activation(out=p, in_=ps_s, func=AF.Exp)
                                nc.vector.tensor_mul(p, p, ez_t.rearrange("p a b -> p (a b)"))
                                p_half.append(p)
                            for kt in range(8):
                                nc.tensor.matmul(
                                    out=ps_pv[base:base + 64, :],
                                    lhsT=vf[kt][:, HDP * h:HDP * (h + 1)],
                                    rhs=p_half[kt // 4][:, 256 * (kt % 4):256 * (kt % 4 + 1)],
                                    start=(h % 2 == 0 and kt == 0),
                                    stop=(h % 2 == 1 and kt == 7),
                                    tile_position=(0, base) if hb else None)
                        # row sums sit at partitions 0 / 64 (ones column of padded V)
                        g, pq = t // 2, t % 2
                        tsum = tmp.tile([128, 256], F32, tag="tsum")
                        nc.vector.tensor_copy(out=tsum[0:1, :], in_=ps_pv[0:1, :])
                        nc.vector.tensor_copy(out=tsum[64:65, :], in_=ps_pv[64:65, :])
                        nc.sync.dma_start(out=S4[g][2 * pq:2 * pq + 1, :], in_=tsum[0:1, :])
                        nc.sync.dma_start(out=S4[g][2 * pq + 1:2 * pq + 2, :], in_=tsum[64:65, :])
                        nc.vector.tensor_mul(xT[t], ps_pv, gate_g[t])
                        if pq == 1:  # group of two pairs done: normalize early
                            nc.vector.reciprocal_approx_fast(out=S4[g], in_=S4[g])
                            nc.vector.tensor_copy(out=R4b[g], in_=S4[g])
                            for tq in (t - 1, t):
                                ps_bc = ps1.tile([128, 256], F32, tag="ps1", name="ps_bc")
                                nc.tensor.matmul(
                                    out=ps_bc, lhsT=sel4[:, g, 128 * (tq % 2):128 * (tq % 2) + 128],
                                    rhs=R4b[g], start=True, stop=True)
                                nc.vector.tensor_mul(xT[tq], xT[tq], ps_bc)

                    # output projection: x = xT.T @ o_w (starts as soon as slots free)
                    ps_x = [ps2.tile([128, D], F32, tag="ps2", name="ps_x") for _ in range(2)]
                    for tq in range(8):
                        wt_ow = wp.tile([128, D], BF16, tag="w768", name="wt_ow")
                        nc.sync.dma_start(out=wt_ow, in_=w_ow[128 * tq:128 * (tq + 1), :])
                        for rt in range(2):
                            for cs in (0, 512):
                                ce = min(cs + 512, D)
                                nc.tensor.matmul(
                                    out=ps_x[rt][:, cs:ce],
                                    lhsT=xT[tq][:, 128 * rt:128 * (rt + 1)],
                                    rhs=wt_ow[:, cs:ce],
                                    start=(tq == 0), stop=(tq == 7))

                    a1_sb = []
                    for rt in range(2):
                        xg = tmp.tile([128, D], BF16, tag="xg")
                        nc.vector.tensor_mul(xg, ps_x[rt], og_sb[rt])
                        a1 = acts.tile([128, D], F32, tag=f"a1_{rt}")
                        nc.vector.tensor_add(a1, a_sb[rt], xg)
                        a1_sb.append(a1)

                    if stage == 3:
                        for rt in range(2):
                            nc.sync.dma_start(out=y_out.ap()[128 * rt:128 * (rt + 1), :], in_=a1_sb[rt])
                    if stage >= 4:
                        # ---------------- AdaLN 2 (sn reused: snw folded on host) --------
                        an2 = [acts.tile([128, D], BF16, tag="lnout", bufs=4, name=f"an2_{rt}") for rt in range(2)]
                        layernorm(an2, a1_sb)
                        a2_sb = [acts.tile([128, D], BF16, tag=f"ba2_{rt}", bufs=1, name=f"a2_{rt}") for rt in range(2)]
                        for rt in range(2):
                            tt = tmp.tile([128, D], BF16, tag="ttmp")
                            nc.vector.tensor_mul(tt, an2[rt], sig2[rt])
                            nc.vector.tensor_add(a2_sb[rt], tt, sb2_sb[rt])
                        a2T = [acts.tile([128, 256], BF16, tag=f"a2T{fc}", name=f"a2T{fc}") for fc in range(FC)]
                        transpose6(a2T, a2_sb, "a2T")

                        # ---------------- transition (feature-on-partition) --------------
                        def proj_convB(w_dram, rhs_tiles, n_oct, wtagbase):
                            # half-width weight chunks: octs 0-5 use half 0, octs 6-11 half 1,
                            # so half-0 slots recycle to the next projection 6 octs earlier.
                            wts = [[None] * FC for _ in range(2)]
                            def load_half(hh):
                                for fc in range(FC):
                                    wt = wcb.tile([128, 768], BF16, tag="wcb768t", bufs=14,
                                                  name=wtagbase)
                                    nc.sync.dma_start(
                                        out=wt,
                                        in_=w_dram[128 * fc:128 * (fc + 1),
                                                        768 * hh:768 * (hh + 1)])
                                    wts[hh][fc] = wt
                            load_half(0)
                            load_half(1)
                            outs = []
                            for t in range(n_oct):
                                hh, tt = t // 6, t % 6
                                ps = ps1.tile([128, 256], F32, tag="ps1", name="ps_cb")
                                for fc in range(FC):
                                    nc.tensor.matmul(out=ps,
                                                     lhsT=wts[hh][fc][:, 128 * tt:128 * (tt + 1)],
                                                     rhs=rhs_tiles[fc],
                                                     start=(fc == 0), stop=(fc == FC - 1))
                                outs.append(ps)
                            return outs

                        # (op gate hoisted to kernel start)
                        hT = [acts.tile([128, 256], BF16, tag=f"hT{t}", name=f"hT{t}") for t in range(12)]
                        u_sb = []
                        for t, ps in enumerate(proj_convB(w_swu, a2T, 12, "wsu")):
                            ut = acts.tile([128, 256], BF16, tag=f"u{t}", name=f"u{t}")
                            nc.vector.tensor_copy(out=ut, in_=ps)
                            u_sb.append(ut)
                        sg_sb = []
                        for t, ps in enumerate(proj_convB(w_swg, a2T, 12, "wsg")):
                            st_ = acts.tile([128, 256], BF16, tag=f"sg{t}", name=f"sg{t}")
                            nc.scalar.activation(out=st_, in_=ps, func=AF.Silu)
                            sg_sb.append(st_)
                        ps_t = [ps2.tile([128, D], F32, tag="ps2", name="ps_t") for _ in range(2)]
                        for t, ps in enumerate(proj_convB(w_ab, a2T, 12, "wab")):
                            hu = tmp.tile([128, 256], BF16, tag="hu")
                            nc.vector.tensor_mul(hu, sg_sb[t], u_sb[t])
                            nc.vector.tensor_mul(hT[t], hu, ps)
                            wt_ba = wp.tile([128, D], BF16, tag="w768", name="wt_ba")
                            nc.sync.dma_start(out=wt_ba, in_=w_ba[128 * t:128 * (t + 1), :])
                            for rt in range(2):
                                for cs in (0, 512):
                                    ce = min(cs + 512, D)
                                    nc.tensor.matmul(
                                        out=ps_t[rt][:, cs:ce],
                                        lhsT=hT[t][:, 128 * rt:128 * (rt + 1)],
                                        rhs=wt_ba[:, cs:ce],
                                        start=(t == 0), stop=(t == 11))

                        for rt in range(2):
                            yt = tmp.tile([128, D], F32, tag="yt", bufs=1)
                            for (hs, he) in ((0, 384), (384, D)):
                                tg = tmp.tile([128, 384], BF16, tag="tg")
                                nc.vector.tensor_mul(
                                    tg, ps_t[rt][:, hs:he], opg_sb[rt][:, hs:he])
                                nc.vector.tensor_add(
                                    yt[:, hs:he], a1_sb[rt][:, hs:he], tg)
                                nc.sync.dma_start(
                                    out=y_out.ap()[128 * rt:128 * (rt + 1), hs:he],
                                    in_=yt[:, hs:he])

    nc.finalize()
    return nc


def _get_nc():
    if "nc" not in _NC_CACHE:
        _NC_CACHE["nc"] = _build_nc()
    return _NC_CACHE["nc"]


def _pad_cols(w):
    """[768, 768] -> [768, 1024]: each head's 48 cols at a 64-aligned block."""
    wp = np.zeros((D, HP), np.float32)
    wp.reshape(D, H, HDP)[:, :, :HD] = np.asarray(w, np.float32).reshape(D, H, HD)
    return wp


def _bf(x):
    return np.ascontiguousarray(np.asarray(x, np.float32).astype(NPBF16))


def kernel(**inputs):
    a = np.asarray(inputs["a"], np.float32)
    s = np.asarray(inputs["s"], np.float32)
    z = np.asarray(inputs["z"], np.float32)

    snw1 = np.asarray(inputs["adaln1_snw"], np.float32)[:, None]
    snw2 = np.asarray(inputs["adaln2_snw"], np.float32)[:, None]
    w_ss1 = _bf(np.vstack([snw1 * np.asarray(inputs["adaln1_ssw"], np.float32),
                           np.asarray(inputs["adaln1_ssb"], np.float32)[None]]))
    w_sb1 = _bf(snw1 * np.asarray(inputs["adaln1_sbw"], np.float32))
    w_ss2 = _bf(np.vstack([snw2 * np.asarray(inputs["adaln2_ssw"], np.float32),
                           np.asarray(inputs["adaln2_ssb"], np.float32)[None]]))
    w_sb2 = _bf(snw2 * np.asarray(inputs["adaln2_sbw"], np.float32))

    w_q = _bf(_pad_cols(inputs["q_w"]) * SCALE)
    qb_p = np.zeros((H, HDP), np.float32)
    qb_p[:, :HD] = np.asarray(inputs["q_b"], np.float32).reshape(H, HD) * SCALE
    qb_p = np.ascontiguousarray(qb_p.reshape(8, 128))
    w_k = _bf(_pad_cols(inputs["k_w"]))
    w_g = _bf(_pad_cols(inputs["g_w"]))
    w_vp = np.zeros((D, HP), np.float32)
    w_vp.reshape(D, H, HDP)[:, :, 1:HD + 1] = \
        np.asarray(inputs["v_w"], np.float32).reshape(D, H, HD)
    w_v = _bf(w_vp)
    w_ow = np.zeros((HP, D), np.float32)
    w_ow.reshape(H, HDP, D)[:, 1:HD + 1, :] = \
        np.asarray(inputs["o_w"], np.float32).reshape(H, HD, D)
    w_ow = _bf(w_ow)
    w_op1 = _bf(np.vstack([np.asarray(inputs["outproj_w"], np.float32),
                           np.asarray(inputs["outproj_b"], np.float32)[None]]))
    w_op2 = _bf(np.vstack([np.asarray(inputs["op_w"], np.float32),
                           np.asarray(inputs["op_b"], np.float32)[None]]))
    sw = np.asarray(inputs["swish_w"], np.float32)
    w_swu = _bf(sw[:, :HID])
    w_swg = _bf(sw[:, HID:])
    w_ab = _bf(inputs["a2b_w"])
    w_ba = _bf(inputs["b2a_w"])

    ident = _bf(np.eye(128))
    sel = np.zeros((4, 4, 2, 128), np.float32)
    for g in range(4):
        for r in range(4):
            for p in range(2):
                for m in range(128):
                    if r == 2 * p + m // 64:
                        sel[g, r, p, m] = 1.0
    sel = _bf(sel.reshape(4, 4, 256).transpose(1, 0, 2))
    vones = np.zeros((1, HP), np.float32)
    vones.reshape(H, HDP)[:, 0] = 1.0
    vones = _bf(vones)

    def f32_bits(x):
        return np.ascontiguousarray(x, np.float32).ravel().view(NPBF16)

    shared = dict(
        ident=ident, sel=sel, vones=vones, qb=f32_bits(qb_p),
        w_ss1=w_ss1, w_sb1=w_sb1, w_ss2=w_ss2, w_sb2=w_sb2,
        w_q=w_q, w_k=w_k, w_g=w_g, w_v=w_v, w_ow=w_ow,
        w_op1=w_op1, w_op2=w_op2, w_swu=w_swu, w_swg=w_swg,
        w_ab=w_ab, w_ba=w_ba,
    )
    # shared tail of the blob (everything after a/qb/s/ez): built once
    tail = np.concatenate(
        [np.asarray(shared[name]).ravel() for name, _, _ in _LAYOUT[4:]]
        , dtype=NPBF16)

    in_maps = []
    for c in range(8):
        beta, q0 = c // 4, 256 * (c % 4)
        rows = slice(q0, q0 + 256)
        ez = np.exp(z[:, beta, rows, :])          # [16, 256, 1024]
        ez = ez.transpose(0, 2, 1)                # [16, 1024k, 256q]
        ez = ez.reshape(H, 8, 128, R).transpose(0, 2, 1, 3)  # [16,128,8,256]
        head = np.concatenate([
            f32_bits(a[beta, rows, :]),
            shared["qb"].ravel(),
            _bf(s[beta, rows, :]).ravel(),
            np.ascontiguousarray(ez.astype(NPBF16)).ravel(),
        ], dtype=NPBF16)
        in_maps.append({"blob": np.concatenate([head, tail], dtype=NPBF16)})

    nc = _get_nc()
    global _LAST_IN_MAPS
    _LAST_IN_MAPS = in_maps
    res = run_bass_kernel_spmd(nc, in_maps, core_ids=list(range(8)))

    out = np.empty((B, N, D), np.float32)
    for c in range(8):
        beta, q0 = c // 4, 256 * (c % 4)
        out[beta, q0:q0 + 256, :] = res.results[c]["y"]
    return out

